# revision 21
# baseline (speedup 1.0000x reference)
"""Trainium2 kernel for nn_Attention_26774826124067.

Math: the reference module's score einsum sums heads out ('bqhe,bkhe->bqk')
and its value einsum sums the key axis out of the probabilities
('bqk,bqhe->bqhe').  Softmax rows sum to 1, so z == V exactly and the whole
module collapses to

    out[b,q,:] = x[b,q,:] @ M + b_O,   M = sum_h W_V[h] @ W_O[h]  (D x D)

independent of W_Q/W_K/b_Q/b_K.  We shard M's columns (and hence output
features) across the 8 NeuronCores: core i computes
    M_i = Wv2 @ Wo2[:, i*256:(i+1)*256]        (2048 x 256)
    outT_i = (x2 @ M_i + b_O_i)^T              (256 x 8192)
with no collectives.  Compute in bf16 (fp32 PSUM accumulation); weights and
activations are pre-transposed/cast on the host so every DMA is a clean
contiguous pattern with >=4KB per-partition descriptors.
"""

import numpy as np
import ml_dtypes

import concourse.bass as bass  # noqa: F401  (engine types come via bacc)
import concourse.bacc as bacc
import concourse.mybir as mybir
from concourse.tile import TileContext
from concourse.bass_utils import run_bass_kernel_spmd

B, S, D, H, DH = 2, 4096, 2048, 16, 128
N_CORES = 8
P = 128
ROWS = B * S              # 8192
COLS = D // N_CORES       # 256 output features per core
KCH = D // P              # 16 contraction chunks (both over d and over h*e)
RB = 512                  # matmul free dim (PSUM bank limit for f32 out)
RB2 = 2048                # row-block (4 matmul slices per block)
N_RB2 = ROWS // RB2       # 4
HS = RB2 // RB            # 4 slices per block
CT = COLS // P            # 2 column tiles of 128 per core

_BF16 = ml_dtypes.bfloat16


def _build_nc():
    f32 = mybir.dt.float32
    bf16 = mybir.dt.bfloat16
    nc = bacc.Bacc(None, target_bir_lowering=False, debug=False)

    xT = nc.declare_dram_parameter("xT", [D, ROWS], bf16, isOutput=False)
    wvT = nc.declare_dram_parameter("wvT", [D, D], bf16, isOutput=False)
    # wo comes pre-swizzled from the host as [P, KCH*COLS]:
    # wo_host[p, k*COLS+n] = Wo2[k*P+p, core_cols[n]] -> contiguous 8KB rows.
    wo = nc.declare_dram_parameter("wo", [P, KCH * COLS], bf16, isOutput=False)
    bo = nc.declare_dram_parameter("bo", [P, CT], f32, isOutput=False)
    out = nc.declare_dram_parameter("out", [COLS, ROWS], bf16, isOutput=True)

    wvT_r = wvT[:].rearrange("(k p) d -> p k d", p=P)  # [128, 16, 2048]

    with TileContext(nc) as tc:
        with (
            tc.tile_pool(name="const", bufs=1) as const_pool,
            tc.tile_pool(name="xb", bufs=1) as x_pool,
            tc.tile_pool(name="ob", bufs=3) as out_pool,
        ):
            wo_sb = const_pool.tile([P, KCH * COLS], bf16)
            bo_sb = const_pool.tile([P, CT], f32)
            nc.scalar.dma_start(out=bo_sb[:], in_=bo[:])
            m_sb = const_pool.tile([P, KCH, COLS], bf16)

            # Stage A: M_i = Wv2 @ Wo2[:, cols].  `start=True` clears the
            # whole PSUM bank, so each accumulation group needs its own bank:
            # process the 16 output d-tiles in two waves of 8.  k (=h*e
            # chunks) stays outermost so PE work starts as soon as the first
            # wvT chunk lands; wvT chunks stay resident in SBUF for wave 2.
            with (
                tc.tile_pool(name="psA", bufs=1, space="PSUM") as psA_pool,
                tc.tile_pool(name="wv", bufs=1) as wv_pool,
            ):
                wvc = [
                    wv_pool.tile([P, D], bf16, name=f"wvc{k}", tag=f"wvc{k}", bufs=1)
                    for k in range(KCH)
                ]
                for k in range(KCH):
                    # interleave so the k=0 matmuls unblock as early as
                    # possible: each k needs wo chunk k + wvT chunk k
                    nc.sync.dma_start(
                        out=wo_sb[:, k * COLS:(k + 1) * COLS],
                        in_=wo[:, k * COLS:(k + 1) * COLS],
                    )
                    nc.sync.dma_start(out=wvc[k][:], in_=wvT_r[:, k, :])
                for wave in range(2):
                    psA = [
                        psA_pool.tile(
                            [P, COLS], f32, name=f"psA{wave}_{j}",
                            tag=f"psA{j}", bufs=1,
                        )
                        for j in range(KCH // 2)
                    ]
                    for k in range(KCH):
                        for j in range(KCH // 2):
                            dtile = wave * (KCH // 2) + j
                            nc.tensor.matmul(
                                psA[j][:],
                                wvc[k][:, dtile * P:(dtile + 1) * P],
                                wo_sb[:, k * COLS:(k + 1) * COLS],
                                start=(k == 0),
                                stop=(k == KCH - 1),
                            )
                    for j in range(KCH // 2):
                        dtile = wave * (KCH // 2) + j
                        if j % 2 == 0:
                            nc.vector.tensor_copy(m_sb[:, dtile, :], psA[j][:])
                        else:
                            nc.scalar.activation(
                                m_sb[:, dtile, :],
                                psA[j][:],
                                mybir.ActivationFunctionType.Identity,
                            )

            # Stage B: outT_i block by block.  x arrives as 16 per-k tiles
            # per 2048-row block (4KB contiguous per partition) on the sync
            # ring, queued behind the weights; outputs leave on the scalar
            # ring.  Eight PSUM accumulation groups (2 col-tiles x 4 row
            # slices) run concurrently; each stationary weight serves four
            # N=512 matmuls.
            with tc.tile_pool(name="psB", bufs=1, space="PSUM") as psB_pool:

                def copy_out(ps, obslice, ct, engine):
                    if engine == 0:
                        nc.vector.tensor_scalar_add(
                            obslice, ps[:], bo_sb[:, ct:ct + 1]
                        )
                    else:
                        nc.scalar.activation(
                            obslice,
                            ps[:],
                            mybir.ActivationFunctionType.Identity,
                            bias=bo_sb[:, ct:ct + 1],
                        )

                for rb in range(N_RB2):
                    xkt = [
                        x_pool.tile(
                            [P, RB2], bf16, name=f"x{rb}_{k}", tag="xkt", bufs=24
                        )
                        for k in range(KCH)
                    ]
                    for k in range(KCH):
                        nc.sync.dma_start(
                            out=xkt[k][:],
                            in_=xT[k * P:(k + 1) * P, rb * RB2:(rb + 1) * RB2],
                        )
                    # The last block runs as two pipelined halves so its
                    # copies/stores overlap matmuls instead of a serial tail.
                    phases = (
                        [range(HS)]
                        if rb < N_RB2 - 1
                        else [range(HS // 2), range(HS // 2, HS)]
                    )
                    for ph, hrange in enumerate(phases):
                        pss = {
                            (ct, h): psB_pool.tile(
                                [P, RB],
                                f32,
                                name=f"ps{rb}_{ph}_{ct}_{h}",
                                tag=f"ps{ct}_{h}",
                                bufs=1,
                            )
                            for ct in range(CT)
                            for h in hrange
                        }
                        for d in range(KCH):
                            for ct in range(CT):
                                for h in hrange:
                                    nc.tensor.matmul(
                                        pss[(ct, h)][:],
                                        m_sb[:, d, ct * P:(ct + 1) * P],
                                        xkt[d][:, h * RB:(h + 1) * RB],
                                        start=(d == 0),
                                        stop=(d == KCH - 1),
                                    )
                        for ct in range(CT):
                            ob = out_pool.tile(
                                [P, len(hrange) * RB],
                                bf16,
                                name=f"ob{rb}_{ph}_{ct}",
                                tag="ob",
                            )
                            for i, h in enumerate(hrange):
                                copy_out(
                                    pss[(ct, h)],
                                    ob[:, i * RB:(i + 1) * RB],
                                    ct,
                                    ct,
                                )
                            c0 = rb * RB2 + hrange[0] * RB
                            nc.scalar.dma_start(
                                out=out[
                                    ct * P:(ct + 1) * P,
                                    c0:c0 + len(hrange) * RB,
                                ],
                                in_=ob[:],
                            )
    nc.compile()
    return nc


_NC = None


def _get_nc():
    global _NC
    if _NC is None:
        _NC = _build_nc()
    return _NC


def prepare_in_maps(normalized_resid_pre, W_V, b_V, W_O, b_O):
    x2 = np.ascontiguousarray(
        np.asarray(normalized_resid_pre, dtype=np.float32).reshape(ROWS, D).T
    ).astype(_BF16)                                        # [D, ROWS]
    wvT = np.ascontiguousarray(
        np.asarray(W_V, dtype=np.float32).transpose(0, 2, 1).reshape(D, D)
    ).astype(_BF16)                                        # [h*e, d]
    # b_V folds into the collapsed matmul as (b_V @ Wo2) added to every row's
    # output; fold it into b_O on the host.
    wo2 = np.asarray(W_O, dtype=np.float32).reshape(D, D)  # [h*e, d']
    bo_full = (
        np.asarray(b_O, dtype=np.float32)
        + np.asarray(b_V, dtype=np.float32).reshape(D) @ wo2
    )                                                      # [D]
    wo_bf = wo2.astype(_BF16)
    in_maps = []
    for i in range(N_CORES):
        cols = slice(i * COLS, (i + 1) * COLS)
        wo_core = (
            wo_bf[:, cols].reshape(KCH, P, COLS).transpose(1, 0, 2).reshape(P, -1)
        )
        in_maps.append(
            {
                "xT": x2,
                "wvT": wvT,
                "wo": np.ascontiguousarray(wo_core),
                "bo": np.ascontiguousarray(
                    bo_full[cols].reshape(CT, P).T
                ),  # [P, CT]
            }
        )
    return in_maps


def assemble_output(results):
    outT = np.concatenate(
        [np.asarray(r["out"]) for r in results], axis=0
    )  # [D, ROWS] bf16, bias already applied on device
    return np.ascontiguousarray(outT.T.astype(np.float32)).reshape(B, S, D)


def kernel(
    normalized_resid_pre,
    W_Q=None,
    b_Q=None,
    W_K=None,
    b_K=None,
    W_V=None,
    b_V=None,
    W_O=None,
    b_O=None,
    **_unused,
):
    nc = _get_nc()
    in_maps = prepare_in_maps(normalized_resid_pre, W_V, b_V, W_O, b_O)
    res = run_bass_kernel_spmd(nc, in_maps, core_ids=list(range(N_CORES)))
    return assemble_output(res.results)


if __name__ == "__main__":
    rng = np.random.default_rng(0)
    x = rng.standard_normal((B, S, D), dtype=np.float32)
    wq = rng.standard_normal((H, D, DH), dtype=np.float32) * 0.02
    wv = rng.standard_normal((H, D, DH), dtype=np.float32) * 0.02
    wo_ = rng.standard_normal((H, DH, D), dtype=np.float32) * 0.02
    out = kernel(
        x,
        W_Q=wq,
        b_Q=np.zeros((H, DH), np.float32),
        W_K=wq,
        b_K=np.zeros((H, DH), np.float32),
        W_V=wv,
        b_V=np.zeros((H, DH), np.float32),
        W_O=wo_,
        b_O=np.zeros((D,), np.float32),
    )
    expect = x.reshape(ROWS, D) @ (
        wv.transpose(1, 0, 2).reshape(D, D) @ wo_.reshape(D, D)
    )
    expect = expect.reshape(B, S, D)
    err = np.abs(out - expect).max() / np.abs(expect).max()
    print("quick self-check rel abs err:", err)
